# revision 9
# baseline (speedup 1.0000x reference)
"""CASCADES adapter (moe_routing) on 8 TRN2 NeuronCores.

Reference computation (B=4, S=2048, D=4096, R=8, K=4):
    centroid[b] = 0.7*x[b,-1] + 0.3*mean_s x[b,s]
    w[b]        = softmax(cos(centroid[b], keys) / 0.05)
    Lam[b]      = sum_k w[b,k] * pool[k]                 # [R,R]
    out[b,s]    = gate * (x[b,s] @ V^T) @ Lam[b]^T @ U^T

Sharding: core i handles batch i//2, sequence half i%2 (1024 rows).
The only cross-core dependency is the full-sequence centroid; each core
contributes 0.3/2048*seqsum_local (+0.7*x_last on odd cores, passed as a
host-prepared aux input) and a pairwise AllReduce of [128,32] (16 KB)
produces the centroid on both cores of each pair.

Everything parameter-only is folded on the host: gate into U, the K
mixing matrices Mk = gate*U @ pool[k] (stacked as Mall^T [32,4096]), and
key normalization. On device the output matmul contracts over 32
partitions: lhsT = w-scaled 4x-replicated x_V^T tile [32,128], rhs =
Mall^T chunk [32,512].
"""

import numpy as np
from contextlib import ExitStack

B, S, D, R, K = 4, 2048, 4096, 8, 4
NCORES = 8
SH = S // 2            # rows per core
PT = 128               # partition tile
NT = SH // PT          # 8 sequence tiles per core
NCH = D // PT          # 32 d-chunks
KR = K * R             # 32

_CACHE = {}
LAST_RESULTS = None


def _build_program():
    from concourse import bacc, tile, mybir

    dt = mybir.dt.float32
    add = mybir.AluOpType.add
    mult = mybir.AluOpType.mult
    AF = mybir.ActivationFunctionType
    AX = mybir.AxisListType

    nc = bacc.Bacc("TRN2", target_bir_lowering=False, debug=False,
                   num_devices=NCORES)

    xs = nc.dram_tensor("xs", [SH, D], dt, kind="ExternalInput").ap()
    vt = nc.dram_tensor("vt", [PT, NCH * KR], dt, kind="ExternalInput").ap()
    mall = nc.dram_tensor("mall", [KR, D], dt, kind="ExternalInput").ap()
    kcols = nc.dram_tensor("kcols", [PT, K * NCH], dt, kind="ExternalInput").ap()
    aux = nc.dram_tensor("aux", [PT, NCH], dt, kind="ExternalInput").ap()
    ident = nc.dram_tensor("ident", [PT, PT], dt, kind="ExternalInput").ap()
    mask = nc.dram_tensor("mask", [KR, K], dt, kind="ExternalInput").ap()
    out = nc.dram_tensor("out", [SH, D], dt, kind="ExternalOutput").ap()

    with tile.TileContext(nc) as tc, ExitStack() as c0:
        persist = c0.enter_context(tc.tile_pool(name="persist", bufs=1))
        dram = c0.enter_context(tc.tile_pool(name="dram", bufs=1, space="DRAM"))

        # ---- constants ----
        vt_sb = persist.tile([PT, NCH, KR], dt, name="vt_sb")
        nc.sync.dma_start(vt_sb[:], vt[:])
        mall_sb = persist.tile([KR, D], dt, name="mall_sb")
        nc.sync.dma_start(mall_sb[:], mall[:])
        kcols_sb = persist.tile([PT, K, NCH], dt, name="kcols_sb")
        nc.sync.dma_start(kcols_sb[:], kcols[:])
        aux_sb = persist.tile([PT, NCH], dt, name="aux_sb")
        nc.sync.dma_start(aux_sb[:], aux[:])
        ident_sb = persist.tile([PT, PT], dt, name="ident_sb")
        nc.sync.dma_start(ident_sb[:], ident[:])
        mask_sb = persist.tile([KR, K], dt, name="mask_sb")
        nc.sync.dma_start(mask_sb[:], mask[:])
        ones_sb = persist.tile([PT, KR], dt, name="ones_sb")
        nc.vector.memset(ones_sb[:], 1.0)

        # ---- persistent intermediates ----
        stash_sb = persist.tile([KR, NT, PT], dt, name="stash_sb")
        seqparts = persist.tile([PT, NCH, NT], dt, name="seqparts")

        # ================= read phase =================
        with ExitStack() as c1:
            xin = c1.enter_context(tc.tile_pool(name="xin", bufs=2))
            xtp = c1.enter_context(
                tc.tile_pool(name="xtp", bufs=3, space="PSUM"))
            xts = c1.enter_context(tc.tile_pool(name="xts", bufs=2))
            xvp = c1.enter_context(
                tc.tile_pool(name="xvp", bufs=2, space="PSUM"))
            xvtp = c1.enter_context(
                tc.tile_pool(name="xvtp", bufs=2, space="PSUM"))
            xvs = c1.enter_context(tc.tile_pool(name="xvs", bufs=2))

            for t in range(NT):
                xtile = xin.tile([PT, D], dt, name="xtile")
                nc.sync.dma_start(xtile[:], xs[t * PT:(t + 1) * PT, :])

                # transpose x tile to d-major, 4 chunks per PSUM bank
                xt_all = xts.tile([PT, NCH, PT], dt, name="xt_all")
                for g in range(NCH // 4):
                    pt_ = xtp.tile([PT, 4, PT], dt, name="pt_")
                    for j in range(4):
                        ch = 4 * g + j
                        nc.tensor.transpose(
                            pt_[:, j, :],
                            xtile[:, ch * PT:(ch + 1) * PT],
                            ident_sb[:],
                        )
                    nc.scalar.copy(xt_all[:, 4 * g:4 * g + 4, :], pt_[:])

                # per-chunk sequence sums for the centroid
                nc.vector.tensor_reduce(
                    seqparts[:, :, t], xt_all[:], axis=AX.X, op=add)

                # x_V accumulation over d-chunks (V columns 4x-replicated
                # on the host, so this lands as [128, 32] directly)
                xv_ps = xvp.tile([PT, KR], dt, name="xv_ps")
                for ch in range(NCH):
                    nc.tensor.matmul(
                        xv_ps[:], xt_all[:, ch, :], vt_sb[:, ch, :],
                        start=(ch == 0), stop=(ch == NCH - 1))
                xv_sb = xvs.tile([PT, KR], dt, name="xv_sb")
                nc.scalar.copy(xv_sb[:], xv_ps[:])

                # transpose -> [32, 128] replica stack
                xvt4 = xvtp.tile([KR, PT], dt, name="xvt4")
                nc.tensor.transpose(xvt4[:], xv_sb[:], ident_sb[:])
                nc.scalar.copy(stash_sb[:, t, :], xvt4[:])

        # ================= routing =================
        cc_sb = persist.tile([PT, NCH], dt, name="cc_sb")
        nc.vector.tensor_reduce(cc_sb[:], seqparts[:], axis=AX.X, op=add)
        nc.vector.tensor_scalar_mul(cc_sb[:], cc_sb[:], 0.3 / S)
        nc.vector.tensor_add(cc_sb[:], cc_sb[:], aux_sb[:])

        cin = dram.tile([PT, NCH], dt, name="cin")
        cout = dram.tile([PT, NCH], dt, name="cout")
        nc.gpsimd.dma_start(cin[:], cc_sb[:])
        nc.gpsimd.collective_compute(
            "AllReduce",
            add,
            replica_groups=[[0, 1], [2, 3], [4, 5], [6, 7]],
            ins=[cin.opt()],
            outs=[cout.opt()],
        )
        c_sb = persist.tile([PT, NCH], dt, name="c_sb")
        nc.gpsimd.dma_start(c_sb[:], cout[:])

        # per-partition partial dots: <c,kn_k> (k=0..3) and |c|^2
        partials = persist.tile([PT, K + 1], dt, name="partials")
        junk = persist.tile([PT, NCH], dt, name="junk")
        for k in range(K):
            nc.vector.tensor_mul(junk[:], c_sb[:], kcols_sb[:, k, :])
            nc.vector.tensor_reduce(
                partials[:, k:k + 1], junk[:], axis=AX.X, op=add)
        nc.vector.tensor_mul(junk[:], c_sb[:], c_sb[:])
        nc.vector.tensor_reduce(
            partials[:, K:K + 1], junk[:], axis=AX.X, op=add)

        with ExitStack() as cm, \
                tc.tile_pool(name="rps", bufs=1, space="PSUM") as rps:
            del cm
            r_ps = rps.tile([KR, K + 1], dt, name="r_ps")
            nc.tensor.matmul(r_ps[:], ones_sb[:], partials[:],
                             start=True, stop=True)
            rt_sb = persist.tile([KR, K + 1], dt, name="rt_sb")
            nc.scalar.copy(rt_sb[:], r_ps[:])

        cn = persist.tile([KR, 1], dt, name="cn")
        nc.scalar.sqrt(cn[:], rt_sb[:, K:K + 1])
        rcn = persist.tile([KR, 1], dt, name="rcn")
        nc.vector.reciprocal(rcn[:], cn[:])
        ex = persist.tile([KR, K], dt, name="ex")
        nc.vector.tensor_scalar(ex[:], rt_sb[:, 0:K], rcn[:], 1.0 / 0.05,
                                op0=mult, op1=mult)
        nc.scalar.activation(ex[:], ex[:], AF.Exp)
        ssum = persist.tile([KR, 1], dt, name="ssum")
        nc.vector.tensor_reduce(ssum[:], ex[:], axis=AX.X, op=add)
        rsum = persist.tile([KR, 1], dt, name="rsum")
        nc.vector.reciprocal(rsum[:], ssum[:])
        wmat = persist.tile([KR, K], dt, name="wmat")
        nc.vector.tensor_scalar_mul(wmat[:], ex[:], rsum[:])
        wcol = persist.tile([KR, 1], dt, name="wcol")
        junk2 = persist.tile([KR, K], dt, name="junk2")
        nc.vector.tensor_mul(junk2[:], wmat[:], mask_sb[:])
        nc.vector.tensor_reduce(wcol[:], junk2[:], axis=AX.X, op=add)

        # ================= write phase =================
        with ExitStack() as c2:
            otp = c2.enter_context(
                tc.tile_pool(name="otp", bufs=4, space="PSUM"))
            osb_pool = c2.enter_context(tc.tile_pool(name="osb", bufs=2))
            xvw_pool = c2.enter_context(tc.tile_pool(name="xvw", bufs=2))

            for t in range(NT):
                xvw = xvw_pool.tile([KR, PT], dt, name="xvw")
                nc.vector.tensor_scalar_mul(
                    xvw[:], stash_sb[:, t, :], wcol[:])
                osb = osb_pool.tile([PT, D], dt, name="osb")
                for n in range(D // 512):
                    o_ps = otp.tile([PT, 512], dt, name="o_ps")
                    nc.tensor.matmul(
                        o_ps[:], xvw[:], mall_sb[:, n * 512:(n + 1) * 512],
                        start=True, stop=True)
                    dst = osb[:, n * 512:(n + 1) * 512]
                    if n % 2 == 0:
                        nc.scalar.copy(dst, o_ps[:])
                    else:
                        nc.vector.tensor_copy(dst, o_ps[:])
                nc.sync.dma_start(out[t * PT:(t + 1) * PT, :], osb[:])

    nc.compile()
    return nc


def _get_program():
    if "nc" not in _CACHE:
        _CACHE["nc"] = _build_program()
    return _CACHE["nc"]


def _host_prep(x, U, V, pool, keys, gate_w, gate_b):
    """Parameter-only folding + per-core shard/aux construction."""
    f32 = np.float32
    # gate (parameter-only)
    gin = np.concatenate([U.mean(axis=0), V.mean(axis=1)]).astype(f32)
    z = gin @ gate_w[0].astype(f32) + gate_b[0].astype(f32)
    gate = f32(1.0) / (f32(1.0) + np.exp(-z, dtype=f32))
    Ug = (gate * U).astype(f32)

    # Mall^T [32, 4096]: rows 8k+j = (gate*U @ pool[k])[:, j]
    mall = np.concatenate(
        [(Ug @ pool[k]).T.astype(f32) for k in range(K)], axis=0)
    mall = np.ascontiguousarray(mall, dtype=f32)

    # V^T in column-chunk layout, replicated 4x along r:
    # [p, c*KR + k*R + r] = V[r, c*128+p]
    vt = np.ascontiguousarray(
        np.tile(V.T.reshape(NCH, PT, R), (1, 1, K))
        .transpose(1, 0, 2).reshape(PT, NCH * KR),
        dtype=f32)

    # normalized keys in column layout [128, K*32]: [p, k*32+c] = kn[k, c*128+p]
    knorm = np.maximum(np.linalg.norm(keys, axis=1, keepdims=True), 1e-8)
    kn = (keys / knorm).astype(f32)
    kcols = np.ascontiguousarray(
        kn.reshape(K, NCH, PT).transpose(2, 0, 1).reshape(PT, K * NCH),
        dtype=f32)

    identity = np.eye(PT, dtype=f32)
    msk = np.zeros((KR, K), dtype=f32)
    for p in range(KR):
        msk[p, p // R] = 1.0

    shared = {"vt": vt, "mall": mall, "kcols": kcols, "ident": identity,
              "mask": msk}

    in_maps = []
    for core in range(NCORES):
        b, h = divmod(core, 2)
        xsrd = np.ascontiguousarray(x[b, h * SH:(h + 1) * SH, :], dtype=f32)
        if h == 1:
            aux = np.ascontiguousarray(
                (f32(0.7) * x[b, S - 1, :]).reshape(NCH, PT).T, dtype=f32)
        else:
            aux = np.zeros((PT, NCH), dtype=f32)
        in_maps.append({"xs": xsrd, "aux": aux, **shared})
    return in_maps


def kernel(x, U_shared, V_shared, core_pool, core_keys, gate_w, gate_b):
    global LAST_RESULTS
    from concourse import bass_utils

    x = np.asarray(x, dtype=np.float32)
    U = np.asarray(U_shared, dtype=np.float32)
    V = np.asarray(V_shared, dtype=np.float32)
    pool = np.asarray(core_pool, dtype=np.float32)
    keys = np.asarray(core_keys, dtype=np.float32)
    gw = np.asarray(gate_w, dtype=np.float32)
    gb = np.asarray(gate_b, dtype=np.float32)

    nc = _get_program()
    in_maps = _host_prep(x, U, V, pool, keys, gw, gb)
    res = bass_utils.run_bass_kernel_spmd(
        nc, in_maps, core_ids=list(range(NCORES)))
    LAST_RESULTS = res

    out = np.empty((B, S, D), dtype=np.float32)
    for core in range(NCORES):
        b, h = divmod(core, 2)
        out[b, h * SH:(h + 1) * SH, :] = res.results[core]["out"]
    return out


# revision 11
# speedup vs baseline: 1.7368x; 1.7368x over previous
"""CASCADES adapter (moe_routing) on 8 TRN2 NeuronCores.

Reference computation (B=4, S=2048, D=4096, R=8, K=4):
    centroid[b] = 0.7*x[b,-1] + 0.3*mean_s x[b,s]
    w[b]        = softmax(cos(centroid[b], keys) / 0.05)
    Lam[b]      = sum_k w[b,k] * pool[k]                 # [R,R]
    out[b,s]    = gate * (x[b,s] @ V^T) @ Lam[b]^T @ U^T

Sharding: core i handles batch i//2, sequence half i%2 (1024 rows).
The only cross-core dependency is the full-sequence centroid; each core
contributes 0.3/2048*seqsum_local (+0.7*x_last on odd cores, passed as a
host-prepared aux input) and a pairwise AllReduce of [128,32] (16 KB)
produces the centroid on both cores of each pair.

Everything parameter-only is folded on the host: gate into U, the K
mixing matrices Mk = gate*U @ pool[k] (stacked as Mall^T [32,4096]), and
key normalization. On device the output matmul contracts over 32
partitions: lhsT = w-scaled 4x-replicated x_V^T tile [32,128], rhs =
Mall^T chunk [32,512].
"""

import numpy as np
from contextlib import ExitStack

B, S, D, R, K = 4, 2048, 4096, 8, 4
NCORES = 8
SH = S // 2            # rows per core
PT = 128               # partition tile
NT = SH // PT          # 8 sequence tiles per core
NCH = D // PT          # 32 d-chunks
KR = K * R             # 32

_CACHE = {}
LAST_RESULTS = None


def _build_program():
    from concourse import bacc, tile, mybir

    dt = mybir.dt.float32
    add = mybir.AluOpType.add
    mult = mybir.AluOpType.mult
    AF = mybir.ActivationFunctionType
    AX = mybir.AxisListType

    nc = bacc.Bacc("TRN2", target_bir_lowering=False, debug=False,
                   num_devices=NCORES)

    xs = nc.dram_tensor("xs", [SH, D], dt, kind="ExternalInput").ap()
    vt = nc.dram_tensor("vt", [PT, NCH * KR], dt, kind="ExternalInput").ap()
    mall = nc.dram_tensor("mall", [KR, D], dt, kind="ExternalInput").ap()
    kcols = nc.dram_tensor("kcols", [PT, K * NCH], dt, kind="ExternalInput").ap()
    aux = nc.dram_tensor("aux", [PT, NCH], dt, kind="ExternalInput").ap()
    ident = nc.dram_tensor("ident", [PT, PT], dt, kind="ExternalInput").ap()
    mask = nc.dram_tensor("mask", [KR, K], dt, kind="ExternalInput").ap()
    out = nc.dram_tensor("out", [SH, D], dt, kind="ExternalOutput").ap()

    f32r = mybir.dt.float32r
    NP_ = NT // 2      # s-tile pairs per core

    with tile.TileContext(nc) as tc, ExitStack() as c0:
        persist = c0.enter_context(tc.tile_pool(name="persist", bufs=1))
        dram = c0.enter_context(tc.tile_pool(name="dram", bufs=1, space="DRAM"))

        # ---- constants ----
        kcols_sb = persist.tile([PT, K, NCH], dt, name="kcols_sb")
        nc.sync.dma_start(kcols_sb[:], kcols[:])
        aux_sb = persist.tile([PT, NCH], dt, name="aux_sb")
        nc.sync.dma_start(aux_sb[:], aux[:])
        ident_sb = persist.tile([PT, PT], dt, name="ident_sb")
        nc.sync.dma_start(ident_sb[:], ident[:])
        mask_sb = persist.tile([KR, K], dt, name="mask_sb")
        nc.sync.dma_start(mask_sb[:], mask[:])
        ones_sb = persist.tile([PT, KR], dt, name="ones_sb")
        nc.vector.memset(ones_sb[:], 1.0)

        # fp32r copies of the matmul weights (ACT copy performs the
        # fp32->fp32r rounding the walrus verifier requires)
        vt_sb = persist.tile([PT, NCH, KR], f32r, name="vt_sb")
        mall_sb = persist.tile([KR, D], f32r, name="mall_sb")
        with tc.tile_pool(name="stage", bufs=1) as stage:
            vt_f32 = stage.tile([PT, NCH * KR], dt, name="vt_f32")
            nc.sync.dma_start(vt_f32[:], vt[:])
            nc.scalar.copy(vt_sb[:], vt_f32[:].rearrange(
                "p (c r) -> p c r", r=KR))
            mall_f32 = stage.tile([KR, D], dt, name="mall_f32")
            nc.sync.dma_start(mall_f32[:], mall[:])
            nc.scalar.copy(mall_sb[:], mall_f32[:])

        # ---- persistent intermediates ----
        stash_sb = persist.tile([KR, NP_, 2 * PT], dt, name="stash_sb")
        seqparts = persist.tile([PT, NCH, NT], dt, name="seqparts")

        # ================= read phase =================
        with ExitStack() as c1:
            xin = c1.enter_context(tc.tile_pool(name="xin", bufs=3))
            xtp = c1.enter_context(
                tc.tile_pool(name="xtp", bufs=3, space="PSUM"))
            xts = c1.enter_context(tc.tile_pool(name="xts", bufs=2))
            xvp = c1.enter_context(
                tc.tile_pool(name="xvp", bufs=2, space="PSUM"))

            for pr in range(NP_):
                # xt_all[p, ch, sub*128+s] = x[pair rows], fp32r, d-major
                xt_all = xts.tile([PT, NCH, 2 * PT], f32r, name="xt_all")
                for sub in range(2):
                    t = 2 * pr + sub
                    xtile = xin.tile([PT, D], dt, name="xtile")
                    nc.sync.dma_start(xtile[:], xs[t * PT:(t + 1) * PT, :])
                    for g in range(NCH // 4):
                        pt_ = xtp.tile([PT, 4, PT], dt, name="pt_")
                        for j in range(4):
                            ch = 4 * g + j
                            nc.tensor.transpose(
                                pt_[:, j, :],
                                xtile[:, ch * PT:(ch + 1) * PT],
                                ident_sb[:],
                            )
                        # rounds fp32 -> fp32r during the PSUM drain
                        nc.scalar.copy(
                            xt_all[:, 4 * g:4 * g + 4,
                                   sub * PT:(sub + 1) * PT],
                            pt_[:])
                        # per-chunk sequence sums (fp32, from PSUM)
                        nc.vector.tensor_reduce(
                            seqparts[:, 4 * g:4 * g + 4, t], pt_[:],
                            axis=AX.X, op=add)

                # x_V^T (4x-replicated rows) for both tiles of the pair:
                # out[kr, sub*128+s], contraction over d in fp32r
                xv_ps = xvp.tile([KR, 2 * PT], dt, name="xv_ps")
                for ch in range(NCH):
                    nc.tensor.matmul(
                        xv_ps[:], vt_sb[:, ch, :], xt_all[:, ch, :],
                        start=(ch == 0), stop=(ch == NCH - 1))
                nc.scalar.copy(stash_sb[:, pr, :], xv_ps[:])

        # ================= routing =================
        cc_sb = persist.tile([PT, NCH], dt, name="cc_sb")
        nc.vector.tensor_reduce(cc_sb[:], seqparts[:], axis=AX.X, op=add)
        nc.vector.tensor_scalar_mul(cc_sb[:], cc_sb[:], 0.3 / S)
        nc.vector.tensor_add(cc_sb[:], cc_sb[:], aux_sb[:])

        cin = dram.tile([PT, NCH], dt, name="cin")
        cout = dram.tile([PT, NCH], dt, name="cout")
        nc.gpsimd.dma_start(cin[:], cc_sb[:])
        nc.gpsimd.collective_compute(
            "AllReduce",
            add,
            replica_groups=[[0, 1], [2, 3], [4, 5], [6, 7]],
            ins=[cin.opt()],
            outs=[cout.opt()],
        )
        c_sb = persist.tile([PT, NCH], dt, name="c_sb")
        nc.gpsimd.dma_start(c_sb[:], cout[:])

        # per-partition partial dots: <c,kn_k> (k=0..3) and |c|^2
        partials = persist.tile([PT, K + 1], dt, name="partials")
        junk = persist.tile([PT, NCH], dt, name="junk")
        for k in range(K):
            nc.vector.tensor_mul(junk[:], c_sb[:], kcols_sb[:, k, :])
            nc.vector.tensor_reduce(
                partials[:, k:k + 1], junk[:], axis=AX.X, op=add)
        nc.vector.tensor_mul(junk[:], c_sb[:], c_sb[:])
        nc.vector.tensor_reduce(
            partials[:, K:K + 1], junk[:], axis=AX.X, op=add)

        with ExitStack() as cm, \
                tc.tile_pool(name="rps", bufs=1, space="PSUM") as rps:
            del cm
            r_ps = rps.tile([KR, K + 1], dt, name="r_ps")
            nc.tensor.matmul(r_ps[:], ones_sb[:], partials[:],
                             start=True, stop=True)
            rt_sb = persist.tile([KR, K + 1], dt, name="rt_sb")
            nc.scalar.copy(rt_sb[:], r_ps[:])

        cn = persist.tile([KR, 1], dt, name="cn")
        nc.scalar.sqrt(cn[:], rt_sb[:, K:K + 1])
        rcn = persist.tile([KR, 1], dt, name="rcn")
        nc.vector.reciprocal(rcn[:], cn[:])
        ex = persist.tile([KR, K], dt, name="ex")
        nc.vector.tensor_scalar(ex[:], rt_sb[:, 0:K], rcn[:], 1.0 / 0.05,
                                op0=mult, op1=mult)
        nc.scalar.activation(ex[:], ex[:], AF.Exp)
        ssum = persist.tile([KR, 1], dt, name="ssum")
        nc.vector.tensor_reduce(ssum[:], ex[:], axis=AX.X, op=add)
        rsum = persist.tile([KR, 1], dt, name="rsum")
        nc.vector.reciprocal(rsum[:], ssum[:])
        wmat = persist.tile([KR, K], dt, name="wmat")
        nc.vector.tensor_scalar_mul(wmat[:], ex[:], rsum[:])
        wcol = persist.tile([KR, 1], dt, name="wcol")
        junk2 = persist.tile([KR, K], dt, name="junk2")
        nc.vector.tensor_mul(junk2[:], wmat[:], mask_sb[:])
        nc.vector.tensor_reduce(wcol[:], junk2[:], axis=AX.X, op=add)

        # ================= write phase =================
        with ExitStack() as c2:
            otp = c2.enter_context(
                tc.tile_pool(name="otp", bufs=4, space="PSUM"))
            osb_pool = c2.enter_context(tc.tile_pool(name="osb", bufs=2))
            xvw_pool = c2.enter_context(tc.tile_pool(name="xvw", bufs=2))

            for t in range(NT):
                xvw = xvw_pool.tile([KR, PT], f32r, name="xvw")
                nc.scalar.mul(
                    xvw[:],
                    stash_sb[:, t // 2, (t % 2) * PT:(t % 2 + 1) * PT],
                    wcol[:])
                osb = osb_pool.tile([PT, D], dt, name="osb")
                for n in range(D // 512):
                    o_ps = otp.tile([PT, 512], dt, name="o_ps")
                    nc.tensor.matmul(
                        o_ps[:], xvw[:], mall_sb[:, n * 512:(n + 1) * 512],
                        start=True, stop=True)
                    dst = osb[:, n * 512:(n + 1) * 512]
                    if n % 2 == 0:
                        nc.scalar.copy(dst, o_ps[:])
                    else:
                        nc.vector.tensor_copy(dst, o_ps[:])
                nc.sync.dma_start(out[t * PT:(t + 1) * PT, :], osb[:])

    nc.compile()
    return nc


def _get_program():
    if "nc" not in _CACHE:
        _CACHE["nc"] = _build_program()
    return _CACHE["nc"]


def _host_prep(x, U, V, pool, keys, gate_w, gate_b):
    """Parameter-only folding + per-core shard/aux construction."""
    f32 = np.float32
    # gate (parameter-only)
    gin = np.concatenate([U.mean(axis=0), V.mean(axis=1)]).astype(f32)
    z = gin @ gate_w[0].astype(f32) + gate_b[0].astype(f32)
    gate = f32(1.0) / (f32(1.0) + np.exp(-z, dtype=f32))
    Ug = (gate * U).astype(f32)

    # Mall^T [32, 4096]: rows 8k+j = (gate*U @ pool[k])[:, j]
    mall = np.concatenate(
        [(Ug @ pool[k]).T.astype(f32) for k in range(K)], axis=0)
    mall = np.ascontiguousarray(mall, dtype=f32)

    # V^T in column-chunk layout, replicated 4x along r:
    # [p, c*KR + k*R + r] = V[r, c*128+p]
    vt = np.ascontiguousarray(
        np.tile(V.T.reshape(NCH, PT, R), (1, 1, K))
        .transpose(1, 0, 2).reshape(PT, NCH * KR),
        dtype=f32)

    # normalized keys in column layout [128, K*32]: [p, k*32+c] = kn[k, c*128+p]
    knorm = np.maximum(np.linalg.norm(keys, axis=1, keepdims=True), 1e-8)
    kn = (keys / knorm).astype(f32)
    kcols = np.ascontiguousarray(
        kn.reshape(K, NCH, PT).transpose(2, 0, 1).reshape(PT, K * NCH),
        dtype=f32)

    identity = np.eye(PT, dtype=f32)
    msk = np.zeros((KR, K), dtype=f32)
    for p in range(KR):
        msk[p, p // R] = 1.0

    shared = {"vt": vt, "mall": mall, "kcols": kcols, "ident": identity,
              "mask": msk}

    in_maps = []
    for core in range(NCORES):
        b, h = divmod(core, 2)
        xsrd = np.ascontiguousarray(x[b, h * SH:(h + 1) * SH, :], dtype=f32)
        if h == 1:
            aux = np.ascontiguousarray(
                (f32(0.7) * x[b, S - 1, :]).reshape(NCH, PT).T, dtype=f32)
        else:
            aux = np.zeros((PT, NCH), dtype=f32)
        in_maps.append({"xs": xsrd, "aux": aux, **shared})
    return in_maps


def kernel(x, U_shared, V_shared, core_pool, core_keys, gate_w, gate_b):
    global LAST_RESULTS
    from concourse import bass_utils

    x = np.asarray(x, dtype=np.float32)
    U = np.asarray(U_shared, dtype=np.float32)
    V = np.asarray(V_shared, dtype=np.float32)
    pool = np.asarray(core_pool, dtype=np.float32)
    keys = np.asarray(core_keys, dtype=np.float32)
    gw = np.asarray(gate_w, dtype=np.float32)
    gb = np.asarray(gate_b, dtype=np.float32)

    nc = _get_program()
    in_maps = _host_prep(x, U, V, pool, keys, gw, gb)
    res = bass_utils.run_bass_kernel_spmd(
        nc, in_maps, core_ids=list(range(NCORES)))
    LAST_RESULTS = res

    out = np.empty((B, S, D), dtype=np.float32)
    for core in range(NCORES):
        b, h = divmod(core, 2)
        out[b, h * SH:(h + 1) * SH, :] = res.results[core]["out"]
    return out
